# revision 21
# baseline (speedup 1.0000x reference)
"""BiLSTM-CRF sequence tagging loss on 8 Trainium2 NeuronCores.

Data-parallel: batch 128 sharded 16/core across 8 cores; each core runs the
full model (embedding gather, 2 BiLSTM layers, FC, CRF forward algorithm)
on its own shard with zero cross-core communication. Host sums the 8
per-core partial losses.

v2 redesign vs the first working kernel:
  - Transposed-state recurrence: hidden/cell state kept as h^T tiles
    [128 gate-rows, 16 batch] so the recurrent matmul needs NO per-step
    transposes and all elementwise work runs on 128 partitions.
  - Gate preactivations are accumulated entirely in PSUM: per 8-step block,
    a bias-broadcast matmul (start=True) then the input-projection matmuls
    and finally the per-step W_hh matmuls (all start=False) add into the
    same PSUM region; tanh reads PSUM directly. No per-step DMA, no
    gate-add on DVE.
  - All large matmuls in bf16 (fp32 PE matmul streams at 1/4 rate).
  - sigmoid-free LSTM cell as before: i,f,o columns pre-scaled by 0.5 so a
    single tanh covers all four gates, with doubled cell/hidden state.
  - Embeddings gathered+transposed once into SBUF (bf16); hcat layers
    round-trip DRAM in the transposed layout (bf16) so layer 1 / FC consume
    them directly as matmul operands.
  - CRF partition function in the exp domain as before; the per-rescale Ln
    calls are batched into one Ln at the end (no ACT table thrash).
"""

import numpy as np

V, E, H, C = 50000, 300, 256, 20
B, T_FULL = 128, 512
N_CORES = 8
B_LOC = B // N_CORES  # 16
G4 = 4 * H  # 1024
TB = 8                 # time steps per block
HDT_FP8 = True         # h-path matmul operands (W_hh, W1i, fcT, h staging, hc) in fp8e4m3
RESCALE_EVERY = 24

_COMPILED = {}


def _build(T, debug=False, phases='ABCDEF', reps=1, nch=4):
    import concourse.bass as bass
    import concourse.mybir as mybir
    import concourse.tile as tile
    from concourse import bacc
    from contextlib import ExitStack

    f32 = mybir.dt.float32
    bf16 = mybir.dt.bfloat16
    hdt = mybir.dt.float8e4 if HDT_FP8 else bf16
    i32 = mybir.dt.int32
    AF = mybir.ActivationFunctionType
    OP = mybir.AluOpType

    NTOK = T * B_LOC            # tokens per core (t-major: tok = t*16 + b)
    NBLK = T // TB              # 8-step blocks
    E_CH = [(0, 128), (128, 128), (256, 44)]
    NRESC = (T - 1) // RESCALE_EVERY  # number of rescales in phase F

    nc = bacc.Bacc("TRN2", debug=False, num_devices=N_CORES)

    def din(name, shape, dt=f32):
        return nc.dram_tensor(name, shape, dt, kind="ExternalInput").ap()

    xt_d = din("xt", (E, NTOK), bf16)   # pre-gathered, pre-transposed embeddings
    w0i_d = din("w0i", (2, E, G4), bf16)
    w0h_d = din("w0h", (2, H, G4), hdt)
    b0g_d = din("b0g", (2, 8, 128), bf16)
    w1i_d = din("w1i", (2, 2 * H, G4), hdt)
    w1h_d = din("w1h", (2, H, G4), hdt)
    b1g_d = din("b1g", (2, 8, 128), bf16)
    ind4_d = din("ind4", (4, 512), bf16)
    fct_d = din("fcT", (2 * H, C), hdt)
    fcb_d = din("fcb", (1, C), bf16)
    ones_d = din("ones1", (1, 128), bf16)
    mask_d = din("maskT", (C, NTOK), bf16)
    pm_d = din("Pm", (C, C))
    est_d = din("estart", (C, 1))
    een_d = din("eend", (C, 1))

    s_out = nc.dram_tensor("S_out", (1, B_LOC), f32, kind="ExternalOutput").ap()
    ne_out = nc.dram_tensor("numE_out", (1, B_LOC), f32, kind="ExternalOutput").ap()
    la_out = nc.dram_tensor("logacc_out", (1, B_LOC), f32, kind="ExternalOutput").ap()

    # DRAM scratch: transposed hidden history [dir, k-chunk, 128, T*16] bf16
    dbg_kind = "ExternalOutput" if debug else "Internal"
    hc0_d = nc.dram_tensor("hc0", (NBLK, 2, 2, 128, 128), hdt, kind=dbg_kind).ap()
    hc1_d = nc.dram_tensor("hc1", (NBLK, 2, 2, 128, 128), hdt, kind=dbg_kind).ap()

    with tile.TileContext(nc) as tc, ExitStack() as top:
        cp = top.enter_context(tc.tile_pool(name="const", bufs=1))

        def load_const(name, dram, shape, dt=f32):
            t = cp.tile(list(shape), dt, tag=name, name=name)
            nc.sync.dma_start(t[:], dram)
            return t

        # weights, transposed layouts, bf16
        w0i_sb = [[load_const(f"w0i{d}_{ki}", w0i_d[d, r0:r0 + ck, :], (ck, G4), bf16)
                   for ki, (r0, ck) in enumerate(E_CH)] for d in range(2)]
        w0h_sb = [[load_const(f"w0h{d}_{k}", w0h_d[d, k * 128:(k + 1) * 128, :], (128, G4), hdt)
                   for k in range(2)] for d in range(2)]
        w1i_sb = [[load_const(f"w1i{d}_{k}", w1i_d[d, k * 128:(k + 1) * 128, :], (128, G4), hdt)
                   for k in range(4)] for d in range(2)]
        w1h_sb = [[load_const(f"w1h{d}_{k}", w1h_d[d, k * 128:(k + 1) * 128, :], (128, G4), hdt)
                   for k in range(2)] for d in range(2)]
        b0g_sb = [[load_const(f"b0g{d}_{jg}", b0g_d[d, jg * 4:(jg + 1) * 4, :], (4, 128), bf16)
                   for jg in range(2)] for d in range(2)]
        b1g_sb = [[load_const(f"b1g{d}_{jg}", b1g_d[d, jg * 4:(jg + 1) * 4, :], (4, 128), bf16)
                   for jg in range(2)] for d in range(2)]
        ind4_sb = load_const("ind4", ind4_d[:], (4, 512), bf16)
        fct_sb = [load_const(f"fct{k}", fct_d[k * 128:(k + 1) * 128, :], (128, C), hdt)
                  for k in range(4)]
        fcb_sb = load_const("fcb", fcb_d[:], (1, C), bf16)
        ones_sb = load_const("ones1", ones_d[:], (1, 128), bf16)
        mask_sb = load_const("maskT", mask_d[:], (C, NTOK), bf16)
        pm_sb = load_const("Pm", pm_d[:], (C, C))
        est_sb = load_const("est", est_d[:], (C, 1))
        een_sb = load_const("een", een_d[:], (C, 1))
        ones20 = cp.tile([C, C], f32)
        nc.vector.memset(ones20[:], 1.0)
        nlnC = cp.tile([C, 1], f32)
        nc.vector.memset(nlnC[:], -float(np.log(C)))

        # embeddings, gathered + transposed once: xT[k] = [128, NTOK] bf16
        xT = [cp.tile([128, NTOK], bf16, tag=f"xT{k}", name=f"xT{k}") for k in range(3)]

        # persistent phase E/F tiles
        ET = cp.tile([C, NTOK], f32)
        numacc = cp.tile([C, NTOK], f32)
        stash = cp.tile([1, max(NRESC, 1) * B_LOC], f32)
        logacc = cp.tile([1, B_LOC], f32)

        def gather_all():
            """Load the host-pre-gathered transposed embeddings."""
            for k, (r0, ck) in enumerate(E_CH):
                nc.sync.dma_start(xT[k][:ck, :], xt_d[r0:r0 + ck, :])

        def recurrence(layer):
            """One BiLSTM layer, both directions interleaved, 8-step blocks."""
            wh_sb = w0h_sb if layer == 0 else w1h_sb
            wi_sb = w0i_sb if layer == 0 else w1i_sb
            bg_sb = b0g_sb if layer == 0 else b1g_sb
            hout_d = hc0_d if layer == 0 else hc1_d

            with ExitStack() as es:
                sp = es.enter_context(tc.tile_pool(name=f"rec{layer}", bufs=3))
                hp = es.enter_context(tc.tile_pool(name=f"rech{layer}", bufs=2))
                pgl = es.enter_context(tc.tile_pool(name=f"pg{layer}", bufs=2, space="PSUM"))

                cT = [cp.tile([128, 32], f32, tag=f"cT{layer}{d}", name=f"cT{layer}{d}") for d in range(2)]
                for d in range(2):
                    nc.vector.memset(cT[d][:], 0.0)

                prev_h = [None, None]   # previous block's hstg tile per dir

                def xg_block(d, blk, pxg):
                    """Bias + input projection for tokens of `blk` into PSUM."""
                    c0 = blk * 128
                    for jg in range(2):
                        nc.tensor.matmul(
                            pxg[:, jg * 512:(jg + 1) * 512],
                            lhsT=bg_sb[d][jg][:],
                            rhs=ind4_sb[:],
                            start=True, stop=False, skip_group_check=True)
                    if layer == 0:
                        for j in range(8):
                            for k, (r0, ck) in enumerate(E_CH):
                                nc.tensor.matmul(
                                    pxg[:, j * 128:(j + 1) * 128],
                                    lhsT=wi_sb[d][k][:ck, j * 128:(j + 1) * 128],
                                    rhs=xT[k][:ck, c0:c0 + 128],
                                    start=False, stop=False, skip_group_check=True)
                    else:
                        hcin = sp.tile([128, 512], hdt, tag=f"hcin{d}", name=f"hcin{d}_{blk}")
                        nc.sync.dma_start(
                            hcin[:], hc0_d[blk].rearrange("a k p c -> p a k c"))
                        for j in range(8):
                            for k in range(4):
                                nc.tensor.matmul(
                                    pxg[:, j * 128:(j + 1) * 128],
                                    lhsT=wi_sb[d][k][:, j * 128:(j + 1) * 128],
                                    rhs=hcin[:, k * 128:(k + 1) * 128],
                                    start=False, stop=False, skip_group_check=True)

                def step(d, t, first, pxg, hstg):
                    """One recurrence step at global time t (slot t%TB)."""
                    sl = t % TB
                    if not first:
                        # rhs = h(prev step) from current/previous staging tile
                        pt = t - 1 if d == 0 else t + 1
                        src = hstg if (pt // TB) == (t // TB) else prev_h[d]
                        psl = pt % TB
                        for j in range(8):
                            for k in range(2):
                                nc.tensor.matmul(
                                    pxg[:, j * 128 + sl * 16: j * 128 + sl * 16 + 16],
                                    lhsT=wh_sb[d][k][:, j * 128:(j + 1) * 128],
                                    rhs=src[:, k * 128 + psl * 16: k * 128 + psl * 16 + 16],
                                    start=False, stop=(k == 1), skip_group_check=True)
                    gv = pxg[:].rearrange("p (j t b) -> p j t b", j=8, t=TB, b=16)
                    Tall = sp.tile([128, 128], f32, tag=f"Tall{d}", name=f"Tall{d}_{t}")
                    # split tanh: i,f,g gates (j=0..5) first so the cell update
                    # starts before the o-gate matmuls (j=6,7) finish
                    nc.scalar.activation(
                        Tall[:, 0:96].rearrange("p (j b) -> p j b", j=6, b=16),
                        gv[:, 0:6, sl, :], AF.Tanh)
                    nc.scalar.activation(
                        Tall[:, 96:128].rearrange("p (j b) -> p j b", j=2, b=16),
                        gv[:, 6:8, sl, :], AF.Tanh)
                    A = sp.tile([128, 32], f32, tag=f"A{d}", name=f"A{d}_{t}")
                    nc.vector.scalar_tensor_tensor(
                        out=A[:], in0=Tall[:, 32:64], scalar=1.0, in1=cT[d][:],
                        op0=OP.add, op1=OP.mult)
                    Bv = sp.tile([128, 32], f32, tag=f"Bv{d}", name=f"Bv{d}_{t}")
                    nc.vector.scalar_tensor_tensor(
                        out=Bv[:], in0=Tall[:, 0:32], scalar=1.0, in1=Tall[:, 64:96],
                        op0=OP.add, op1=OP.mult)
                    nc.vector.scalar_tensor_tensor(
                        out=cT[d][:], in0=A[:], scalar=0.5, in1=Bv[:],
                        op0=OP.mult, op1=OP.add)
                    TC = sp.tile([128, 32], f32, tag=f"TC{d}", name=f"TC{d}_{t}")
                    nc.scalar.activation(TC[:], cT[d][:], AF.Tanh, scale=0.5)
                    hv = hstg[:].rearrange("p (k t b) -> p k t b", k=2, t=TB, b=16)[:, :, sl, :]
                    nc.vector.scalar_tensor_tensor(
                        out=hv, in0=Tall[:, 96:128], scalar=1.0,
                        in1=TC[:].rearrange("p (k b) -> p k b", k=2, b=16),
                        op0=OP.add, op1=OP.mult)

                for bi in range(NBLK):
                    blk = [bi, NBLK - 1 - bi]   # fwd ascending, bwd descending
                    pxg = [pgl.tile([128, 8 * 128], f32, tag=f"pxg{d}", name=f"pxg{d}_{bi}")
                           for d in range(2)]
                    hstg = [hp.tile([128, 2 * TB * 16], hdt, tag=f"hstg{d}", name=f"hstg{d}_{bi}")
                            for d in range(2)]
                    for d in range(2):
                        xg_block(d, blk[d], pxg[d])
                    for i in range(TB):
                        for d in range(2):
                            t = blk[d] * TB + (i if d == 0 else TB - 1 - i)
                            first = (bi == 0 and i == 0)
                            step(d, t, first, pxg[d], hstg[d])
                    for d in range(2):
                        nc.sync.dma_start(
                            hout_d[blk[d], d].rearrange("k p c -> p k c"),
                            hstg[d][:])
                        prev_h[d] = hstg[d]

        def emissions(pef):
            with ExitStack() as es:
                sp = es.enter_context(tc.tile_pool(name="pE", bufs=3))
                for m in range(NBLK):
                    c0 = m * 128
                    hcin = sp.tile([128, 512], hdt, tag="hcin1", name=f"hc1_{m}")
                    nc.sync.dma_start(
                        hcin[:], hc1_d[m].rearrange("a k p c -> p a k c"))
                    ps = pef.tile([128, 128], f32, tag="aux", name=f"emT{m}")
                    nc.tensor.matmul(ps[:C, :], lhsT=fcb_sb[:], rhs=ones_sb[:],
                                     start=True, stop=False, skip_group_check=True)
                    for k in range(4):
                        nc.tensor.matmul(
                            ps[:C, :], lhsT=fct_sb[k][:], rhs=hcin[:, k * 128:(k + 1) * 128],
                            start=False, stop=(k == 3), skip_group_check=True)
                    # numerator: emissions at the gold labels, via mask multiply
                    nc.vector.scalar_tensor_tensor(
                        out=numacc[:, c0:c0 + 128], in0=ps[:C, :], scalar=0.0,
                        in1=mask_sb[:, c0:c0 + 128], op0=OP.add, op1=OP.mult)
                    # exp(e - ln C) transposed for the CRF
                    nc.scalar.activation(ET[:, c0:c0 + 128], ps[:C, :], AF.Exp,
                                         bias=nlnC[:, :1])
                # numE[b] = sum_c sum_t numacc[c, t*16+b]
                nred = sp.tile([C, B_LOC], f32, tag="nred")
                nc.vector.tensor_reduce(
                    nred[:].rearrange("p (b o) -> p b o", o=1),
                    numacc[:].rearrange("p (t b) -> p b t", t=T, b=B_LOC),
                    axis=mybir.AxisListType.X, op=OP.add)
                psn = pef.tile([1, B_LOC], f32, tag="fps", name="psnum")
                nc.tensor.matmul(psn[:], lhsT=ones20[:, :1], rhs=nred[:],
                                 start=True, stop=True)
                neo = sp.tile([1, B_LOC], f32, tag="neo")
                nc.scalar.copy(out=neo[:], in_=psn[:])
                nc.sync.dma_start(ne_out[:], neo[:])

        def crf(pef):
            NCH = nch                    # independent sample-chains (DVE overhead vs latency sweet spot)
            W = B_LOC // NCH
            with ExitStack() as es:
                sp = es.enter_context(tc.tile_pool(name="pCRF", bufs=4))
                nc.vector.memset(logacc[:], 0.0)
                a = []
                for ch in range(NCH):
                    ac = sp.tile([C, W], f32, tag=f"a0_{ch}", name=f"a0_{ch}")
                    nc.vector.tensor_scalar(ac[:], ET[:, ch * W:(ch + 1) * W],
                                            est_sb[:, :1], None, op0=OP.mult)
                    a.append(ac)
                nresc = 0
                for t in range(1, T):
                    resc = (t % RESCALE_EVERY == 0)
                    for ch in range(NCH):
                        ps = pef.tile([C, W], f32, tag="fps", name=f"psa{t}_{ch}")
                        nc.tensor.matmul(ps[:], lhsT=pm_sb[:], rhs=a[ch][:], start=True, stop=True)
                        ac = sp.tile([C, W], f32, tag=f"a{t % 3 + 1}_{ch}", name=f"a{t}_{ch}")
                        nc.vector.scalar_tensor_tensor(
                            out=ac[:], in0=ps[:], scalar=0.0,
                            in1=ET[:, t * B_LOC + ch * W: t * B_LOC + (ch + 1) * W],
                            op0=OP.add, op1=OP.mult)
                        a[ch] = ac
                    if resc:
                        for ch in range(NCH):
                            nrm = pef.tile([C, W], f32, tag="fps", name=f"nrm{t}_{ch}")
                            nc.tensor.matmul(nrm[:], lhsT=ones20[:], rhs=a[ch][:], start=True, stop=True)
                            nc.scalar.copy(
                                out=stash[:, nresc * B_LOC + ch * W: nresc * B_LOC + (ch + 1) * W],
                                in_=nrm[:1, :])
                            rcp = sp.tile([C, W], f32, tag=f"rcp{ch}", name=f"rcp{t}_{ch}")
                            nc.vector.reciprocal(rcp[:], nrm[:])
                            a2 = sp.tile([C, W], f32, tag=f"ars{ch}", name=f"ars{t}_{ch}")
                            nc.vector.tensor_tensor(out=a2[:], in0=a[ch][:], in1=rcp[:], op=OP.mult)
                            a[ch] = a2
                        nresc += 1
                for ch in range(NCH):
                    af = sp.tile([C, W], f32, tag=f"af{ch}", name=f"af{ch}")
                    nc.vector.tensor_scalar(af[:], a[ch][:], een_sb[:, :1], None, op0=OP.mult)
                    pss = pef.tile([1, W], f32, tag="fps", name=f"pss{ch}")
                    nc.tensor.matmul(pss[:], lhsT=ones20[:, :1], rhs=af[:], start=True, stop=True)
                    so = sp.tile([1, W], f32, tag=f"so{ch}", name=f"so{ch}")
                    nc.scalar.copy(out=so[:], in_=pss[:])
                    nc.sync.dma_start(s_out[:, ch * W:(ch + 1) * W], so[:])
                # logacc[b] = sum_r ln(stash[r, b]) -- one batched Ln
                lns = sp.tile([1, NRESC * B_LOC], f32, tag="lns")
                nc.scalar.activation(lns[:], stash[:, :NRESC * B_LOC], AF.Ln)
                nc.vector.tensor_reduce(
                    logacc[:].rearrange("p (b o) -> p b o", o=1),
                    lns[:].rearrange("p (r b) -> p b r", r=NRESC, b=B_LOC),
                    axis=mybir.AxisListType.X, op=OP.add)
                nc.sync.dma_start(la_out[:], logacc[:])

        def whole_model():
            if 'A' in phases:
                gather_all()
            if 'B' in phases:
                recurrence(0)
            if 'D' in phases:
                recurrence(1)
            if 'E' in phases or 'F' in phases:
                with ExitStack() as es:
                    pef = es.enter_context(tc.tile_pool(name="pEF", bufs=2, space="PSUM"))
                    if 'E' in phases:
                        emissions(pef)
                    if 'F' in phases:
                        crf(pef)

        for _rep in range(reps):
            whole_model()

    nc.compile()
    return nc


def _prep_host(inputs, T):
    """Host-side weight transforms + per-core in_maps."""
    import ml_dtypes
    f32 = np.float32
    bf16 = ml_dtypes.bfloat16
    NTOK = T * B_LOC
    NBLK = T // TB
    ids_full = np.asarray(inputs["input_ids"]).astype(np.int32)      # [B, T]
    labels = np.asarray(inputs["labels"]).astype(np.int64)           # [B, T]
    emb = np.asarray(inputs["emb"], dtype=f32).astype(bf16)
    trans = np.asarray(inputs["transitions"], dtype=f32)
    start = np.asarray(inputs["start_trans"], dtype=f32)
    end = np.asarray(inputs["end_trans"], dtype=f32)

    colscale = np.ones(G4, f32)
    colscale[0:256] = 0.5       # i
    colscale[256:512] = 0.5     # f
    colscale[768:1024] = 0.5    # o

    hdt = ml_dtypes.float8_e4m3 if HDT_FP8 else bf16

    def prep_layer(wi, wh, bi, bh, in_scale, wi_dt):
        wiT = np.ascontiguousarray(np.transpose(wi, (0, 2, 1))).astype(f32)
        whT = np.ascontiguousarray(np.transpose(wh, (0, 2, 1))).astype(f32)
        wiT = wiT * in_scale * colscale[None, None, :]
        whT = whT * 0.5 * colscale[None, None, :]
        b = (np.asarray(bi, f32) + np.asarray(bh, f32)) * colscale[None, :]
        return wiT.astype(wi_dt), whT.astype(hdt), b.reshape(2, 8, 128).astype(bf16)

    w0i, w0h, b0g = prep_layer(inputs["w_ih_l0"], inputs["w_hh_l0"],
                               inputs["b_ih_l0"], inputs["b_hh_l0"], 1.0, bf16)
    w1i, w1h, b1g = prep_layer(inputs["w_ih_l1"], inputs["w_hh_l1"],
                               inputs["b_ih_l1"], inputs["b_hh_l1"], 0.5, hdt)
    fcT = (np.asarray(inputs["fc_w"], f32).T * 0.5).astype(hdt)     # [2H, C]
    fcb = np.asarray(inputs["fc_b"], f32).reshape(1, C).astype(bf16)
    ind4 = (np.arange(512)[None, :] // 128 == np.arange(4)[:, None]).astype(bf16)
    ones1 = np.ones((1, 128), bf16)
    Pm = np.exp(trans).astype(f32)
    est = np.exp(start).astype(f32).reshape(C, 1)
    een = np.exp(end).astype(f32).reshape(C, 1)

    in_maps = []
    host_num = np.zeros(B, np.float64)
    for c in range(N_CORES):
        bs = slice(c * B_LOC, (c + 1) * B_LOC)
        ids_c = ids_full[bs, :T].T.reshape(NTOK)                     # (t,b) t-major
        xt = np.ascontiguousarray(emb[ids_c].T)                      # [E, NTOK] bf16
        lab_c = labels[bs, :T].T.reshape(NTOK)                       # token (t,b)
        maskT = np.zeros((C, NTOK), f32)
        maskT[lab_c, np.arange(NTOK)] = 1.0
        in_maps.append({
            "xt": xt,
            "w0i": w0i, "w0h": w0h, "b0g": b0g,
            "w1i": w1i, "w1h": w1h, "b1g": b1g,
            "ind4": ind4, "fcT": fcT, "fcb": fcb, "ones1": ones1,
            "maskT": maskT.astype(bf16),
            "Pm": Pm, "estart": est, "eend": een,
        })
        lb = labels[bs, :T]
        host_num[c * B_LOC:(c + 1) * B_LOC] = (
            start[lb[:, 0]].astype(np.float64)
            + trans[lb[:, :-1], lb[:, 1:]].sum(-1)
            + end[lb[:, -1]]
        )
    return in_maps, host_num


def _run(inputs, T):
    from concourse.bass_utils import run_bass_kernel_spmd

    if T not in _COMPILED:
        _COMPILED[T] = _build(T)
    nc = _COMPILED[T]
    in_maps, host_num = _prep_host(inputs, T)
    res = run_bass_kernel_spmd(nc, in_maps, core_ids=list(range(N_CORES)))
    total = 0.0
    for c in range(N_CORES):
        r = res.results[c]
        S = r["S_out"].reshape(B_LOC).astype(np.float64)
        numE = r["numE_out"].reshape(B_LOC).astype(np.float64)
        logacc = r["logacc_out"].reshape(B_LOC).astype(np.float64)
        logZ = np.log(S) + logacc + T * np.log(C)
        num = host_num[c * B_LOC:(c + 1) * B_LOC] + numE
        total += (logZ - num).sum()
    return np.float32(total)


def kernel(**inputs):
    return _run(inputs, T_FULL)

